# revision 9
# baseline (speedup 1.0000x reference)
"""Trainium2 Bass kernel for nn_AttentionProb (GNN edge attention probs).

Computes att[e] = clamp(sigmoid(score_src[src[e]] + score_dst[dst[e]]))
where score_src = x @ w_src, score_dst = x @ w_dst.

Strategy (8 NeuronCores, SPMD):
  - Edge-parallel: each core handles 200k of the 1.6M edges.
  - x is row-sharded (12500 nodes/core); each core computes its slice of the
    score tables on the PE, then an AllGather replicates the full tables.
  - Gather: score tables live in SBUF as 16 channels per Q7 core
    ({src,dst} x 8 node-chunks of 12500).  nc.gpsimd.ap_gather fetches the
    16 chunk-candidates per edge; a DVE is_equal mask (against host-sent
    chunk ids) + multiply zeroes the wrong candidates; a block-diagonal PE
    matmul reduces 16->1 across partitions and sums the src+dst streams in
    PSUM; ACT applies sigmoid; DVE clamps; DMA out.
"""

import numpy as np

import concourse.bass as bass
import concourse.bacc as bacc
import concourse.mybir as mybir
import concourse.tile as tile
from concourse import library_config
from concourse.bass_utils import run_bass_kernel_spmd

# ---------------------------------------------------------------- constants
N_CORES = 8
NUM_NODES = 100000
D = 128
NUM_EDGES = 1600000
CLAMP_MIN = 1e-05
CLAMP_MAX = 0.99999

NODES_PC = NUM_NODES // N_CORES          # 12500 nodes per core
EDGES_PC = NUM_EDGES // N_CORES          # 200000 edges per core
N_GROUPS = 8                             # Q7 cores (16 partitions each)
E_G = 25088                              # edges per Q7 group (padded)
E_PAD = N_GROUPS * E_G                   # 200704 padded edges per core
CHUNK = NODES_PC                         # node-table chunk per channel
IDX_COLS = E_G // 16                     # 1568 int16 index columns

# j-chunk sizes for the gather loop (multiples of 512 for clean matmuls)
J_CHUNKS = [2048] * 12 + [512]
assert sum(J_CHUNKS) == E_G

F32 = mybir.dt.float32
I16 = mybir.dt.int16
U8 = mybir.dt.uint8


def build_program(mm_n=500):
    """Build the SPMD Bass program (same for all cores)."""
    nc = bacc.Bacc("TRN2", target_bir_lowering=False)

    xt = nc.dram_tensor("xt", [D, NODES_PC], F32, kind="ExternalInput")
    w2 = nc.dram_tensor("w2", [D, 2], F32, kind="ExternalInput")
    osrc = nc.dram_tensor("osrc", [128, IDX_COLS], I16, kind="ExternalInput")
    odst = nc.dram_tensor("odst", [128, IDX_COLS], I16, kind="ExternalInput")
    csrc = nc.dram_tensor("csrc", [128, E_G], U8, kind="ExternalInput")
    cdst = nc.dram_tensor("cdst", [128, E_G], U8, kind="ExternalInput")
    cpart = nc.dram_tensor("cpart", [128, 1], F32, kind="ExternalInput")
    sel = nc.dram_tensor("sel", [128, N_GROUPS], F32, kind="ExternalInput")
    att = nc.dram_tensor("att", [N_GROUPS, E_G], F32, kind="ExternalOutput")

    with tile.TileContext(nc) as tc:
        with (
            tc.tile_pool(name="const", bufs=1) as const,
            tc.tile_pool(name="dram", bufs=1, space="DRAM") as dram,
            tc.tile_pool(name="table", bufs=1) as table_pool,
        ):
            # ---------------- phase A: local scores = w2.T @ xt ----------
            wt = const.tile([D, 2], F32)
            nc.sync.dma_start(wt[:], w2[:])
            cpart_sb = const.tile([128, 1], F32)
            nc.sync.dma_start(cpart_sb[:], cpart[:])
            sel_sb = const.tile([128, N_GROUPS], F32)
            nc.sync.dma_start(sel_sb[:], sel[:])
            osrc_sb = const.tile([128, IDX_COLS], I16)
            nc.sync.dma_start(osrc_sb[:], osrc[:])
            odst_sb = const.tile([128, IDX_COLS], I16)
            nc.sync.dma_start(odst_sb[:], odst[:])

            agin = dram.tile([2, NODES_PC], F32)
            with (
                tc.tile_pool(name="phA", bufs=3) as phA,
                tc.tile_pool(name="psA", bufs=2, space="PSUM") as psA,
            ):
                scores = phA.tile([2, NODES_PC], F32, tag="scores")
                n_mm = NODES_PC // mm_n
                assert NODES_PC % mm_n == 0 and mm_n <= 512
                for i in range(n_mm):
                    sx = phA.tile([D, mm_n], F32, tag="sx")
                    nc.sync.dma_start(sx[:], xt[:, i * mm_n:(i + 1) * mm_n])
                    ps = psA.tile([2, mm_n], F32)
                    nc.tensor.matmul(ps[:], wt[:], sx[:], start=True, stop=True)
                    nc.scalar.activation(
                        scores[:, i * mm_n:(i + 1) * mm_n], ps[:],
                        mybir.ActivationFunctionType.Copy)
                nc.sync.dma_start(agin[:], scores[:])
            agout = dram.tile([2 * N_CORES, NODES_PC], F32)
            nc.gpsimd.collective_compute(
                "AllGather",
                mybir.AluOpType.bypass,
                replica_groups=[list(range(N_CORES))],
                ins=[agin.opt()],
                outs=[agout.opt()],
            )

            # table: partition 16q + 8h + c  holds stream h, node chunk c
            # agout row layout: row 2c + h  (core c's [src; dst] slices)
            TT = table_pool.tile([128, CHUNK], F32)
            ag_v = agout[:].rearrange("(c h) n -> c h n", c=N_CORES, h=2)
            for q in range(N_GROUPS):
                nc.sync.dma_start(TT[16 * q:16 * q + 8, :], ag_v[:, 0, :])
                nc.sync.dma_start(TT[16 * q + 8:16 * q + 16, :], ag_v[:, 1, :])

            # ---------------- phase B: gather + select + sigmoid ---------
            with (
                tc.tile_pool(name="cbuf", bufs=3) as cbuf,
                tc.tile_pool(name="gbuf", bufs=2) as gbuf,
                tc.tile_pool(name="mbuf", bufs=2) as mbuf,
                tc.tile_pool(name="obuf", bufs=2) as obuf,
                tc.tile_pool(name="psB", bufs=2, space="PSUM") as psB,
            ):
                off = 0
                for J in J_CHUNKS:
                    cs = cbuf.tile([128, J], U8, tag="cs")
                    nc.sync.dma_start(cs[:], csrc[:, off:off + J])
                    cd = cbuf.tile([128, J], U8, tag="cd")
                    nc.sync.dma_start(cd[:], cdst[:, off:off + J])

                    gs = gbuf.tile([128, J], F32, tag="gs")
                    nc.gpsimd.ap_gather(
                        gs[:], TT[:], osrc_sb[:, off // 16:(off + J) // 16],
                        channels=128, num_elems=CHUNK, d=1, num_idxs=J)
                    gd = gbuf.tile([128, J], F32, tag="gd")
                    nc.gpsimd.ap_gather(
                        gd[:], TT[:], odst_sb[:, off // 16:(off + J) // 16],
                        channels=128, num_elems=CHUNK, d=1, num_idxs=J)

                    ms = mbuf.tile([128, J], F32, tag="ms")
                    nc.vector.tensor_scalar(
                        ms[:], cs[:], cpart_sb[:], None,
                        mybir.AluOpType.is_equal)
                    md = mbuf.tile([128, J], F32, tag="md")
                    nc.vector.tensor_scalar(
                        md[:], cd[:], cpart_sb[:], None,
                        mybir.AluOpType.is_equal)
                    # P = G * M (in place over M)
                    nc.vector.tensor_tensor(
                        ms[:], gs[:], ms[:], mybir.AluOpType.mult)
                    nc.vector.tensor_tensor(
                        md[:], gd[:], md[:], mybir.AluOpType.mult)

                    ps = psB.tile([N_GROUPS, J], F32)
                    for i in range(J // 512):
                        sl = slice(512 * i, 512 * (i + 1))
                        nc.tensor.matmul(
                            ps[:, sl], sel_sb[:], ms[:, sl],
                            start=True, stop=False)
                        nc.tensor.matmul(
                            ps[:, sl], sel_sb[:], md[:, sl],
                            start=False, stop=True)

                    ob = obuf.tile([N_GROUPS, J], F32, tag="ob")
                    nc.scalar.activation(
                        ob[:], ps[:], mybir.ActivationFunctionType.Sigmoid)
                    nc.vector.tensor_scalar(
                        ob[:], ob[:], CLAMP_MIN, CLAMP_MAX,
                        mybir.AluOpType.max, mybir.AluOpType.min)
                    nc.sync.dma_start(att[:, off:off + J], ob[:])
                    off += J

    nc.finalize()
    return nc


def prep_inputs(x, edge_index, att_weight):
    """Host-side shard + layout prep. Returns per-core input maps."""
    x = np.asarray(x, dtype=np.float32)
    edge_index = np.asarray(edge_index)
    att_weight = np.asarray(att_weight, dtype=np.float32)

    w2 = np.ascontiguousarray(att_weight.reshape(2, D).T)   # [128, 2]
    cpart = np.ascontiguousarray(
        np.tile(np.arange(16, dtype=np.float32), N_GROUPS)[:, None])
    sel = np.zeros((128, N_GROUPS), dtype=np.float32)
    for q in range(N_GROUPS):
        sel[16 * q:16 * (q + 1), q] = 1.0

    in_maps = []
    for n in range(N_CORES):
        xs = x[n * NODES_PC:(n + 1) * NODES_PC]             # [12500, 128]
        xt = np.ascontiguousarray(xs.T)                     # [128, 12500]

        es = edge_index[:, n * EDGES_PC:(n + 1) * EDGES_PC].astype(np.int64)
        src = np.zeros(E_PAD, dtype=np.int64)
        dst = np.zeros(E_PAD, dtype=np.int64)
        src[:EDGES_PC] = es[0]
        dst[:EDGES_PC] = es[1]
        src = src.reshape(N_GROUPS, E_G)
        dst = dst.reshape(N_GROUPS, E_G)

        o_src = (src % CHUNK).astype(np.int16)
        o_dst = (dst % CHUNK).astype(np.int16)
        c_src = (src // CHUNK).astype(np.uint8)             # 0..7
        c_dst = (dst // CHUNK).astype(np.uint8) + 8         # 8..15

        # idx wrap: partition 16q+p, col s  <-  o[q, 16s+p]
        osrc_t = o_src.reshape(N_GROUPS, IDX_COLS, 16).transpose(0, 2, 1)
        odst_t = o_dst.reshape(N_GROUPS, IDX_COLS, 16).transpose(0, 2, 1)
        osrc_t = np.ascontiguousarray(osrc_t).reshape(128, IDX_COLS)
        odst_t = np.ascontiguousarray(odst_t).reshape(128, IDX_COLS)

        # chunk ids replicated across the 16 partitions of each group
        csrc_r = np.repeat(c_src[:, None, :], 16, axis=1).reshape(128, E_G)
        cdst_r = np.repeat(c_dst[:, None, :], 16, axis=1).reshape(128, E_G)

        in_maps.append({
            "xt": xt,
            "w2": w2,
            "osrc": osrc_t,
            "odst": odst_t,
            "csrc": np.ascontiguousarray(csrc_r),
            "cdst": np.ascontiguousarray(cdst_r),
            "cpart": cpart,
            "sel": sel,
        })
    return in_maps


_PROGRAM_CACHE = {}


def _get_program():
    if "nc" not in _PROGRAM_CACHE:
        _PROGRAM_CACHE["nc"] = build_program()
    return _PROGRAM_CACHE["nc"]


def run(x, edge_index, att_weight, trace=False, trace_kwargs=None, tmpdir=None):
    """Run on 8 NeuronCores; returns (att[NUM_EDGES] f32, BassKernelResults)."""
    nc = _get_program()
    in_maps = prep_inputs(x, edge_index, att_weight)
    kw = {}
    if trace:
        kw = dict(trace=True, trace_kwargs=trace_kwargs or {}, tmpdir=tmpdir)
    res = run_bass_kernel_spmd(nc, in_maps, core_ids=list(range(N_CORES)), **kw)
    out = np.empty(NUM_EDGES, dtype=np.float32)
    for n in range(N_CORES):
        flat = res.results[n]["att"].reshape(E_PAD)
        out[n * EDGES_PC:(n + 1) * EDGES_PC] = flat[:EDGES_PC]
    return out, res


def kernel(x, edge_index, att_weight):
    out, _ = run(x, edge_index, att_weight)
    return out
